# revision 19
# baseline (speedup 1.0000x reference)
"""Trainium2 Bass kernel for nn_Encoder_74182675137046.

Reference computation (per image of 1024x1024 complex pixels):
    feats = [norm_row, norm_col, x0, x1]  per pixel     [N, 4]
    h   = relu((feats @ W1 + b1) @ W2 + b2)             [N, 128]
    out = h @ W3 + b3                                   [N, 128]
    result = (w * out).sum(0) / w.sum()                 [128]
with w = (x0 != 0), and norm_row/col normalized by masked min/max.

Algebraic folding (exact):
    fc1+fc2 fold:  h_pre = feats @ W12 + b12,  W12 = W1@W2, b12 = b1@W2 + b2
    pool/fc3 swap: (w*out).sum = (sum_p w_p*relu(h_pre_p)) @ W3 + w.sum()*b3
So the device only computes S = sum_p relu(h_pre_p)  (a [128] vector per
core); the tiny [128]x[128,128] tail runs on host in float64.

Device design (per core, 128 image rows = 131072 points):
    - rhs tiles [128p, 2048] hold 4 chunks at partition groups {32g+k}:
      row 32g+0 = x0, 32g+1 = x1, 32g+2 = norm_col pattern, 32g+3 = ones.
      One DMA per feature row per fill (DMA dest APs stride partitions only
      in their first dim).
    - Per image row r: h.T[128d, 1024] = lhsT_r[4,128].T @ rhs[4,1024] via two
      N=512 fp32 matmuls, row-group tiled (tile_position=(32g,0)) so the four
      groups' matmuls run concurrently in distinct 32-row strips of the PE.
      lhsT row 3 = b12 + nr_r*W12[0] (per-image-row), so PSUM holds the full
      pre-relu activation and consumers need no bias.
    - Consumers alternate ScalarE activation(Relu, accum_out) and VectorE
      tensor_scalar(op0=max(.,0), op1=add-reduce, accum_out): one pass over
      PSUM producing per-row [128,1] partial sums (the free-dim reduction is
      free), written in-place to PSUM to avoid SBUF scratch WAW chains.
    - TPB instructions have one sync-wait slot; tiny observer matmuls and
      same-engine carrier ops (plus per-proc pre-drains) keep every
      instruction at <=1 semaphore wait.
"""

import numpy as np

import concourse.bass as bass
import concourse.tile as tile
from concourse import mybir
from concourse.bass_utils import run_bass_kernel_spmd
from concourse.tile_rust import add_dep_helper

H = 1024
W = 1024
D = 128
N_CORES = 8
ROWS_PER_CORE = H // N_CORES          # 128
NPTS = ROWS_PER_CORE * W              # 131072
CHUNK = 2 * W                         # 2048 pts per (group, fill) = 2 image rows
NGROUPS = 4
FILL_PTS = NGROUPS * CHUNK            # 8192
NFILLS = NPTS // FILL_PTS             # 16
NCHUNKS = NPTS // CHUNK               # 64
NT = 512                              # fp32 matmul moving-operand max free dim
NROWFEAT = 4                          # x0, x1, ncol, ones

F32 = mybir.dt.float32

TRACE = False
LAST_RESULT = None

_NC_CACHE = None


def _build_bass():
    """Build the SPMD Bass program (same program on all 8 cores)."""
    global _NC_CACHE
    if _NC_CACHE is not None:
        return _NC_CACHE

    nc = bass.Bass()

    # Per-core inputs. xd rows per chunk: (x0, x1, norm_col, ones).
    xd = nc.dram_tensor("xd", [NCHUNKS, NROWFEAT, CHUNK], F32, kind="ExternalInput")
    # lwt[32g+k, 128*s : 128*(s+1)] = lhsT row k for group g, slot s=2f+half:
    # k=0,1,2 -> W12[2], W12[3], W12[1]; k=3 -> b12 + nr_r*W12[0].
    lwt = nc.dram_tensor("lwt", [128, 128 * 2 * NFILLS], F32, kind="ExternalInput")
    outs = nc.dram_tensor("outs", [128, 1], F32, kind="ExternalOutput")

    with tile.TileContext(nc) as tc:
        with (
            tc.tile_pool(name="singles", bufs=1) as singles,
            tc.tile_pool(name="psall", bufs=1, space="PSUM") as psall,
        ):
            lw_t = singles.tile([128, 128 * 2 * NFILLS], F32)
            lw_dma = nc.sync.dma_start(out=lw_t[:], in_=lwt[:])

            red = singles.tile([128, ROWS_PER_CORE], F32)
            tiny_a = singles.tile([128, 1], F32)
            tiny_v = singles.tile([128, 1], F32)
            tiny_v2 = singles.tile([128, 1], F32)

            # One rhs slot per fill: fill DMAs have no WAR/WAW deps at all.
            rhs_all = singles.tile([128, NFILLS, CHUNK], F32)
            ps_all = psall.tile([128, NGROUPS, W], F32)     # 4 slots x 2 banks

            op_idx = 0
            prev_act = []   # previous half's ACT consumers
            prev_dve = []   # previous half's DVE consumers
            fill_dmas = []
            last_mm = None
            for f in range(NFILLS):
                rhs = rhs_all[:, f, :]
                rg = rhs.rearrange("(g r) c -> g r c", r=32)

                # One DMA per feature row k (dest partition stride 32).
                for k in range(NROWFEAT):
                    dma = nc.sync.dma_start(
                        out=rg[:, k, :],
                        in_=xd[NGROUPS * f : NGROUPS * (f + 1), k, :],
                    )
                    fill_dmas.append(dma)

                # Matmult instructions get a single sync-wait slot in walrus.
                # One observer matmul absorbs the g0-slot consumer release
                # (or the lwt DMA on fill 0) via its psum-cell WAR; then a
                # K=1..4 cascade (all based at partition 0) absorbs each fill
                # DMA lane. Real matmuls then need at most one wait.
                obs_src = lw_t[0:4, 0:1]
                nc.tensor.matmul(
                    ps_all[0:1, 0, 0:1],
                    obs_src,
                    obs_src,
                    start=True,
                    stop=True,
                    tile_position=(0, 0),
                )
                for k in range(NROWFEAT):
                    nc.tensor.matmul(
                        ps_all[0:1, 0, 0:1],
                        rhs[0 : k + 1, 0:1],
                        rhs[0 : k + 1, 0:1],
                        start=True,
                        stop=True,
                        tile_position=(0, 0),
                    )

                for half in range(2):
                    s = 2 * f + half
                    # Interleave groups so LDWEIGHTS overlaps across row groups.
                    for j in range(W // NT):
                        for g in range(NGROUPS):
                            col0 = half * W + j * NT
                            last_mm = nc.tensor.matmul(
                                ps_all[:, g, j * NT : (j + 1) * NT],
                                lw_t[32 * g : 32 * g + 4, 128 * s : 128 * (s + 1)],
                                rhs[32 * g : 32 * g + 4, col0 : col0 + NT],
                                start=True,
                                stop=True,
                                tile_position=(32 * g, 0),
                            )

                    # Per-half same-engine "carrier" ops absorb the in-place
                    # PSUM WAW ticks of the previous half's consumers, so each
                    # real consumer carries only its single PE wait.
                    if prev_act:
                        ca = nc.scalar.activation(
                            out=tiny_a[:],
                            in_=red[:, 0:1],
                            func=mybir.ActivationFunctionType.Relu,
                        )
                        for ins in prev_act:
                            add_dep_helper(ca.ins, ins, reason="ACT observes prev")
                    if prev_dve:
                        cv = nc.vector.tensor_scalar(
                            out=tiny_v[:],
                            in0=red[:, 2:3],
                            scalar1=0.0,
                            scalar2=None,
                            op0=mybir.AluOpType.add,
                        )
                        for ins in prev_dve:
                            add_dep_helper(cv.ins, ins, reason="DVE observes prev")

                    cur_act, cur_dve = [], []
                    for g in range(NGROUPS):
                        r_loc = 8 * f + 2 * g + half  # local image row index
                        acc_ap = red[:, r_loc : r_loc + 1]
                        span = ps_all[:, g, :]
                        if op_idx % 2 == 0:
                            cons = nc.scalar.activation(
                                out=span,
                                in_=span,
                                func=mybir.ActivationFunctionType.Relu,
                                accum_out=acc_ap,
                            )
                            cur_act.append(cons.ins)
                        else:
                            cons = nc.vector.tensor_scalar(
                                out=span,
                                in0=span,
                                scalar1=0.0,
                                scalar2=None,
                                op0=mybir.AluOpType.max,
                                op1=mybir.AluOpType.add,
                                accum_out=acc_ap,
                            )
                            cur_dve.append(cons.ins)
                        op_idx += 1
                    prev_act, prev_dve = cur_act, cur_dve

            # Final DVE carrier observes ACT's tail so the reduce needs one
            # wait (red col 0 is ACT-written, merging into the same sem).
            cfin = nc.vector.tensor_scalar(
                out=tiny_v2[:],
                in0=red[:, 0:1],
                scalar1=0.0,
                scalar2=None,
                op0=mybir.AluOpType.add,
            )
            for ins in prev_act:
                add_dep_helper(cfin.ins, ins, reason="reduce observes ACT tail")
            outs_t = singles.tile([128, 1], F32)
            rsum = nc.vector.reduce_sum(outs_t[:], red[:], axis=mybir.AxisListType.X)
            # SWDGE: lands on a fresh software-DMA lane, so the only wait is
            # the reduce's DVE tick (TPB DMA descriptors take a single wait).
            odma = nc.gpsimd.dma_start(out=outs[:], in_=outs_t[:])

            # The TileContext-exit drain gets every unobserved proc's sem as a
            # wait, but a Drain lowers to a single-wait NOP. Pre-observe each
            # proc with its own single-wait drain on SP.
            drain_deps = [prev_act[-1], rsum.ins, last_mm.ins, odma.ins, lw_dma.ins]
            drain_deps += [d.ins for d in fill_dmas[-8:]]
            for dins in drain_deps:
                dr = nc.sync.drain(fusable=False)
                add_dep_helper(dr.ins, dins, reason="pre-drain observe")

    _NC_CACHE = nc
    return nc


def kernel(x, W1, b1, W2, b2, W3, b3):
    global LAST_RESULT
    x = np.asarray(x, dtype=np.float32)
    W1 = np.asarray(W1, dtype=np.float32)
    b1 = np.asarray(b1, dtype=np.float32)
    W2 = np.asarray(W2, dtype=np.float32)
    b2 = np.asarray(b2, dtype=np.float32)
    W3 = np.asarray(W3, dtype=np.float32)
    b3 = np.asarray(b3, dtype=np.float32)

    x0, x1 = x[0], x[1]
    mask = x0 != 0.0

    # Masked min/max of row/col indices (reference semantics).
    rows_any = mask.any(axis=1)
    cols_any = mask.any(axis=0)
    ridx = np.nonzero(rows_any)[0]
    cidx = np.nonzero(cols_any)[0]
    rmin, rmax = float(ridx[0]), float(ridx[-1])
    cmin, cmax = float(cidx[0]), float(cidx[-1])

    W12 = W1.astype(np.float64) @ W2.astype(np.float64)        # [4, 128]
    b12 = b1.astype(np.float64) @ W2.astype(np.float64) + b2   # [128]
    v0 = W12[0]

    nr_all = (np.arange(H, dtype=np.float64) - rmin) / (rmax - rmin)
    nc_all = (np.arange(W, dtype=np.float64) - cmin) / (cmax - cmin)

    ncrow = np.tile(nc_all.astype(np.float32), 2)               # [2048]
    v2f = W12[2].astype(np.float32)
    v3f = W12[3].astype(np.float32)
    v1f = W12[1].astype(np.float32)

    nc_prog = _build_bass()
    in_maps = []
    for c in range(N_CORES):
        shard = x[:, c * ROWS_PER_CORE : (c + 1) * ROWS_PER_CORE, :]
        xdv = np.empty((NCHUNKS, NROWFEAT, CHUNK), dtype=np.float32)
        xdv[:, 0, :] = shard[0].reshape(NCHUNKS, CHUNK)
        xdv[:, 1, :] = shard[1].reshape(NCHUNKS, CHUNK)
        xdv[:, 2, :] = ncrow[None, :]
        xdv[:, 3, :] = 1.0
        # Per-row bias table folded into lhsT row 3.
        btab = (
            b12[:, None]
            + np.outer(v0, nr_all[c * ROWS_PER_CORE : (c + 1) * ROWS_PER_CORE])
        ).astype(np.float32)  # [128 dims, 128 local rows]
        lwtv = np.zeros((128, 128 * 2 * NFILLS), dtype=np.float32)
        for g in range(NGROUPS):
            for s in range(2 * NFILLS):
                f, half = divmod(s, 2)
                r_loc = 8 * f + 2 * g + half
                blk = slice(128 * s, 128 * (s + 1))
                lwtv[32 * g + 0, blk] = v2f
                lwtv[32 * g + 1, blk] = v3f
                lwtv[32 * g + 2, blk] = v1f
                lwtv[32 * g + 3, blk] = btab[:, r_loc]
        in_maps.append({"xd": xdv, "lwt": lwtv})

    res = run_bass_kernel_spmd(
        nc_prog, in_maps, core_ids=list(range(N_CORES)), trace=TRACE
    )
    LAST_RESULT = res

    S = np.zeros(D, dtype=np.float64)
    for c in range(N_CORES):
        S += res.results[c]["outs"][:, 0].astype(np.float64)

    # Correction: device summed relu over ALL points; reference sums only
    # masked (x0 != 0) points. Subtract the (rare/empty) unmasked set exactly.
    if not mask.all():
        zr, zc = np.nonzero(~mask)
        hz = (
            np.outer(nr_all[zr], W12[0])
            + np.outer(nc_all[zc], W12[1])
            + np.outer(x1[zr, zc].astype(np.float64), W12[3])
            + b12[None, :]
        )  # x0 contribution is exactly zero for these points
        S -= np.maximum(hz, 0.0).sum(axis=0)

    wsum = float(mask.sum())
    out = (S @ W3.astype(np.float64)) / wsum + b3.astype(np.float64)
    return out.astype(np.float32)


# revision 32
# speedup vs baseline: 1.1435x; 1.1435x over previous
"""Trainium2 Bass kernel for nn_Encoder_74182675137046.

Reference computation (per image of 1024x1024 complex pixels):
    feats = [norm_row, norm_col, x0, x1]  per pixel     [N, 4]
    h   = relu((feats @ W1 + b1) @ W2 + b2)             [N, 128]
    out = h @ W3 + b3                                   [N, 128]
    result = (w * out).sum(0) / w.sum()                 [128]
with w = (x0 != 0), and norm_row/col normalized by masked min/max.

Algebraic folding (exact):
    fc1+fc2 fold:  h_pre = feats @ W12 + b12,  W12 = W1@W2, b12 = b1@W2 + b2
    pool/fc3 swap: (w*out).sum = (sum_p w_p*relu(h_pre_p)) @ W3 + w.sum()*b3
So the device only computes S = sum_p relu(h_pre_p)  (a [128] vector per
core); the tiny [128]x[128,128] tail runs on host in float64.

Device design (per core, 128 image rows = 131072 points): see the build
function. Key points: four tile_position row groups with K=4 feature rows
(x0, x1, norm_col, ones) at partitions {32g+k}; float32r single-pass
matmuls with the per-image-row bias folded into lhsT row 3; consumers
read adjacent PSUM slot pairs as FD=2048 relu+accумulate ops split between
ScalarE and VectorE; per-fill rhs slots (no reuse -> dep-free DMAs) with
prefilled constant rows; observer matmuls / narrow carriers keep every
instruction within the single TPB sync-wait slot.
"""

import numpy as np

import concourse.bass as bass
import concourse.tile as tile
from concourse import mybir
from concourse.bass_utils import run_bass_kernel_spmd
from concourse.tile_rust import add_dep_helper

H = 1024
W = 1024
D = 128
N_CORES = 8
ROWS_PER_CORE = H // N_CORES          # 128
NPTS = ROWS_PER_CORE * W              # 131072
CHUNK = 2 * W                         # 2048 pts per (group, fill) = 2 image rows
NGROUPS = 4
FILL_PTS = NGROUPS * CHUNK            # 8192
NFILLS = NPTS // FILL_PTS             # 16
NT = 512
ROW_SLOTS = 2 * NFILLS                # lhsT blocks per group (32)

F32 = mybir.dt.float32
F32R = mybir.dt.float32r

TRACE = False
LAST_RESULT = None

_NC_CACHE = None


def _build_bass():
    """Build the SPMD Bass program (same program on all 8 cores)."""
    global _NC_CACHE
    if _NC_CACHE is not None:
        return _NC_CACHE

    nc = bass.Bass()

    xd = nc.dram_tensor("xd", [NFILLS, 2, NGROUPS, CHUNK], F32R,
                        kind="ExternalInput")
    lwt = nc.dram_tensor("lwt", [128, 128 * ROW_SLOTS], F32R,
                         kind="ExternalInput")
    ncpre = nc.dram_tensor("ncpre", [NGROUPS, 8 * CHUNK], F32R,
                           kind="ExternalInput")
    onepre = nc.dram_tensor("onepre", [NGROUPS, 8 * CHUNK], F32R,
                            kind="ExternalInput")
    outs = nc.dram_tensor("outs", [128, 1], F32, kind="ExternalOutput")

    with tile.TileContext(nc) as tc:
        with (
            tc.tile_pool(name="singles", bufs=1) as singles,
            tc.tile_pool(name="psall", bufs=1, space="PSUM") as psall,
        ):
            lw_t = singles.tile([128, 128 * ROW_SLOTS], F32R)
            lw_dma = nc.sync.dma_start(out=lw_t[:], in_=lwt[:])

            red = singles.tile([128, 4 * NFILLS], F32)
            tiny_a = singles.tile([128, 1], F32)
            tiny_v = singles.tile([128, 1], F32)
            tiny_v2 = singles.tile([128, 1], F32)

            rhs_all = singles.tile([128, NFILLS, CHUNK], F32R)
            ps_all = psall.tile([128, NGROUPS, W], F32)

            # Prefill constant rows (norm_col k=2, ones k=3) across all
            # slots, split in two chunks so fill 0 is not gated on the tail.
            rall = rhs_all.rearrange("(g r) s c -> g r (s c)", r=32)
            pres = []
            for lo in range(2):
                cols = slice(8 * CHUNK * lo, 8 * CHUNK * (lo + 1))
                pres.append(nc.sync.dma_start(out=rall[:, 2, cols], in_=ncpre[:]))
                pres.append(nc.sync.dma_start(out=rall[:, 3, cols], in_=onepre[:]))

            last_act = None
            last_dve = None
            fill_dmas = []
            last_mm = None
            for f in range(NFILLS):
                rhs = rhs_all[:, f, :]
                rg = rhs.rearrange("(g r) c -> g r c", r=32)

                # x0/x1 rows: partitions {32g+k}, dep-free (fresh slot).
                for k in range(2):
                    dma = nc.sync.dma_start(
                        out=rg[:, k, :],
                        in_=xd[f, k, :, :],
                    )
                    fill_dmas.append(dma)

                def make_carriers():
                    # Narrow per-engine carriers for the in-place WAW ticks;
                    # the j=0 matmuls hang their slot-WAR dep off them, so
                    # carrier -> matmul -> consumer order is structural.
                    ca = cv = None
                    if last_act is not None:
                        ca = nc.scalar.activation(
                            out=tiny_a[:], in_=red[:, 0:1],
                            func=mybir.ActivationFunctionType.Relu,
                        )
                        add_dep_helper(ca.ins, last_act, reason="ACT WAW carry")
                    if last_dve is not None:
                        cv = nc.vector.tensor_scalar(
                            out=tiny_v[:], in0=red[:, 1:2],
                            scalar1=0.0, scalar2=None, op0=mybir.AluOpType.add,
                        )
                        add_dep_helper(cv.ins, last_dve, reason="DVE WAW carry")
                    return ca, cv

                # Half-0 carriers come BEFORE the observers: the observer
                # chain rides the ACT carrier, so observers and matmuls have
                # equal readiness and emission priority puts observers first.
                ca, cv = make_carriers()

                # Observer matmuls: the first absorbs the slot-0 psum cell
                # WAR (merged with the ACT carrier tick; lwt DMA at f=0);
                # the K=1..2 cascade absorbs the x0/x1 DMA lanes; K=3..4 at
                # f in {0,8} also absorb the prefill chunk lanes.
                obs_src = lw_t[0:4, 0:1].bitcast(F32)
                obs = nc.tensor.matmul(
                    ps_all[0:1, 0, 0:1], obs_src, obs_src,
                    start=True, stop=True, tile_position=(0, 0),
                )
                if ca is not None:
                    add_dep_helper(obs.ins, ca.ins, reason="obs after carrier")
                kmax = 4 if f in (0, 8) else 2
                last_obs = None
                for k in range(kmax):
                    cobs = nc.tensor.matmul(
                        ps_all[0:1, 0, 0:1],
                        rhs[0 : k + 1, 0:1].bitcast(F32),
                        rhs[0 : k + 1, 0:1].bitcast(F32),
                        start=True, stop=True, tile_position=(0, 0),
                    )
                    if ca is not None:
                        add_dep_helper(cobs.ins, ca.ins, reason="obs after carrier")
                    last_obs = cobs

                for half in range(2):
                    s = 2 * f + half
                    if half == 1:
                        ca, cv = make_carriers()

                    # Interleave groups so LDWEIGHTS overlaps across groups.
                    for j in range(2):
                        for g in range(NGROUPS):
                            col0 = half * W + j * NT
                            last_mm = nc.tensor.matmul(
                                ps_all[:, g, j * NT : (j + 1) * NT],
                                lw_t[32 * g : 32 * g + 4,
                                     128 * s : 128 * (s + 1)],
                                rhs[32 * g : 32 * g + 4, col0 : col0 + NT],
                                start=True, stop=True,
                                tile_position=(32 * g, 0),
                            )
                            # Every matmul (j=0 and j=1) rides the carrier
                            # and observer chain so none is hoisted above
                            # them (j=1 has its own slot-WAR range dep too).
                            carrier = ca if g < 2 else cv
                            if carrier is not None:
                                add_dep_helper(
                                    last_mm.ins, carrier.ins,
                                    reason="slot WAR via carrier",
                                )
                            add_dep_helper(
                                last_mm.ins, last_obs.ins,
                                reason="mm after observers",
                            )

                    # Consumers: ACT owns slots {0,1}, DVE owns {2,3}; each
                    # reads its adjacent slot pair as one FD=2048 op.
                    i_red = 4 * f + 2 * half
                    cons_a = nc.scalar.activation(
                        out=ps_all[:, 0:2, :],
                        in_=ps_all[:, 0:2, :],
                        func=mybir.ActivationFunctionType.Relu,
                        accum_out=red[:, i_red : i_red + 1],
                    )
                    last_act = cons_a.ins
                    cons_v = nc.vector.tensor_scalar(
                        out=ps_all[:, 2:4, :],
                        in0=ps_all[:, 2:4, :],
                        scalar1=0.0,
                        scalar2=None,
                        op0=mybir.AluOpType.max,
                        op1=mybir.AluOpType.add,
                        accum_out=red[:, i_red + 1 : i_red + 2],
                    )
                    last_dve = cons_v.ins

            # Final DVE carrier observes ACT's tail so the reduce needs one
            # wait (red col 0 is ACT-written: merges into the same sem).
            cfin = nc.vector.tensor_scalar(
                out=tiny_v2[:], in0=red[:, 0:1],
                scalar1=0.0, scalar2=None, op0=mybir.AluOpType.add,
            )
            add_dep_helper(cfin.ins, last_act, reason="reduce observes ACT tail")
            outs_t = singles.tile([128, 1], F32)
            rsum = nc.vector.reduce_sum(outs_t[:], red[:],
                                        axis=mybir.AxisListType.X)
            # SWDGE: separate lane space -> single wait (the reduce).
            odma = nc.gpsimd.dma_start(out=outs[:], in_=outs_t[:])

            # Pre-observe every proc on SP so the TileContext-exit drain
            # (single-wait NOP) has nothing left to wait on.
            drain_deps = [last_act, rsum.ins, last_mm.ins, odma.ins,
                          lw_dma.ins]
            drain_deps += [p.ins for p in pres]
            drain_deps += [d.ins for d in fill_dmas[-8:]]
            for dins in drain_deps:
                dr = nc.sync.drain(fusable=False)
                add_dep_helper(dr.ins, dins, reason="pre-drain observe")

    _NC_CACHE = nc
    return nc


def kernel(x, W1, b1, W2, b2, W3, b3):
    global LAST_RESULT
    x = np.asarray(x, dtype=np.float32)
    W1 = np.asarray(W1, dtype=np.float32)
    b1 = np.asarray(b1, dtype=np.float32)
    W2 = np.asarray(W2, dtype=np.float32)
    b2 = np.asarray(b2, dtype=np.float32)
    W3 = np.asarray(W3, dtype=np.float32)
    b3 = np.asarray(b3, dtype=np.float32)

    x0, x1 = x[0], x[1]
    mask = x0 != 0.0

    rows_any = mask.any(axis=1)
    cols_any = mask.any(axis=0)
    ridx = np.nonzero(rows_any)[0]
    cidx = np.nonzero(cols_any)[0]
    rmin, rmax = float(ridx[0]), float(ridx[-1])
    cmin, cmax = float(cidx[0]), float(cidx[-1])

    W12 = W1.astype(np.float64) @ W2.astype(np.float64)
    b12 = b1.astype(np.float64) @ W2.astype(np.float64) + b2
    v0 = W12[0]

    nr_all = (np.arange(H, dtype=np.float64) - rmin) / (rmax - rmin)
    nc_all = (np.arange(W, dtype=np.float64) - cmin) / (cmax - cmin)

    nc2 = np.tile(nc_all.astype(np.float32), 2)
    v2f = W12[2].astype(np.float32)
    v3f = W12[3].astype(np.float32)
    v1f = W12[1].astype(np.float32)

    ncpre = np.broadcast_to(
        np.tile(nc2, 8)[None, :], (NGROUPS, 8 * CHUNK)
    ).copy()
    onepre = np.ones((NGROUPS, 8 * CHUNK), dtype=np.float32)

    nc_prog = _build_bass()
    in_maps = []
    for c in range(N_CORES):
        shard = x[:, c * ROWS_PER_CORE : (c + 1) * ROWS_PER_CORE, :]
        xdv = np.empty((NFILLS, 2, NGROUPS, CHUNK), dtype=np.float32)
        sh = shard.reshape(2, NFILLS, NGROUPS, CHUNK)
        xdv[:, 0, :, :] = sh[0]
        xdv[:, 1, :, :] = sh[1]

        btab = (
            b12[:, None]
            + np.outer(v0, nr_all[c * ROWS_PER_CORE : (c + 1) * ROWS_PER_CORE])
        ).astype(np.float32)
        lwtv = np.zeros((128, 128 * ROW_SLOTS), dtype=np.float32)
        for g in range(NGROUPS):
            for s in range(ROW_SLOTS):
                f, half = divmod(s, 2)
                r_loc = 8 * f + 2 * g + half
                blk = slice(128 * s, 128 * (s + 1))
                lwtv[32 * g + 0, blk] = v2f
                lwtv[32 * g + 1, blk] = v3f
                lwtv[32 * g + 2, blk] = v1f
                lwtv[32 * g + 3, blk] = btab[:, r_loc]
        in_maps.append({"xd": xdv, "lwt": lwtv, "ncpre": ncpre,
                        "onepre": onepre})

    res = run_bass_kernel_spmd(
        nc_prog, in_maps, core_ids=list(range(N_CORES)), trace=TRACE
    )
    LAST_RESULT = res

    S = np.zeros(D, dtype=np.float64)
    for c in range(N_CORES):
        S += res.results[c]["outs"][:, 0].astype(np.float64)

    if not mask.all():
        zr, zc = np.nonzero(~mask)
        hz = (
            np.outer(nr_all[zr], W12[0])
            + np.outer(nc_all[zc], W12[1])
            + np.outer(x1[zr, zc].astype(np.float64), W12[3])
            + b12[None, :]
        )
        S -= np.maximum(hz, 0.0).sum(axis=0)

    wsum = float(mask.sum())
    out = (S @ W3.astype(np.float64)) / wsum + b3.astype(np.float64)
    return out.astype(np.float32)


# revision 33
# speedup vs baseline: 1.1781x; 1.0303x over previous
"""Trainium2 Bass kernel for nn_Encoder_74182675137046.

Reference computation (per image of 1024x1024 complex pixels):
    feats = [norm_row, norm_col, x0, x1]  per pixel     [N, 4]
    h   = relu((feats @ W1 + b1) @ W2 + b2)             [N, 128]
    out = h @ W3 + b3                                   [N, 128]
    result = (w * out).sum(0) / w.sum()                 [128]
with w = (x0 != 0), and norm_row/col normalized by masked min/max.

Algebraic folding (exact):
    fc1+fc2 fold:  h_pre = feats @ W12 + b12,  W12 = W1@W2, b12 = b1@W2 + b2
    pool/fc3 swap: (w*out).sum = (sum_p w_p*relu(h_pre_p)) @ W3 + w.sum()*b3
So the device only computes S = sum_p relu(h_pre_p)  (a [128] vector per
core); the tiny [128]x[128,128] tail runs on host in float64.

Device design (per core, 128 image rows = 131072 points): see the build
function. Key points: four tile_position row groups with K=4 feature rows
(x0, x1, norm_col, ones) at partitions {32g+k}; float32r single-pass
matmuls with the per-image-row bias folded into lhsT row 3; consumers
read adjacent PSUM slot pairs as FD=2048 relu+accумulate ops split between
ScalarE and VectorE; per-fill rhs slots (no reuse -> dep-free DMAs) with
prefilled constant rows; observer matmuls / narrow carriers keep every
instruction within the single TPB sync-wait slot.
"""

import numpy as np

import concourse.bass as bass
import concourse.tile as tile
from concourse import mybir
from concourse.bass_utils import run_bass_kernel_spmd
from concourse.tile_rust import add_dep_helper

H = 1024
W = 1024
D = 128
N_CORES = 8
ROWS_PER_CORE = H // N_CORES          # 128
NPTS = ROWS_PER_CORE * W              # 131072
CHUNK = 2 * W                         # 2048 pts per (group, fill) = 2 image rows
NGROUPS = 4
FILL_PTS = NGROUPS * CHUNK            # 8192
NFILLS = NPTS // FILL_PTS             # 16
NT = 512
ROW_SLOTS = 2 * NFILLS                # lhsT blocks per group (32)

F32 = mybir.dt.float32
F32R = mybir.dt.float32r

TRACE = False
LAST_RESULT = None

_NC_CACHE = None


def _build_bass():
    """Build the SPMD Bass program (same program on all 8 cores)."""
    global _NC_CACHE
    if _NC_CACHE is not None:
        return _NC_CACHE

    nc = bass.Bass()

    xd = nc.dram_tensor("xd", [NFILLS, 2, NGROUPS, CHUNK], F32R,
                        kind="ExternalInput")
    lwt = nc.dram_tensor("lwt", [128, 128 * ROW_SLOTS], F32R,
                         kind="ExternalInput")
    ncpre = nc.dram_tensor("ncpre", [NGROUPS, 8 * CHUNK], F32R,
                           kind="ExternalInput")
    onepre = nc.dram_tensor("onepre", [NGROUPS, 8 * CHUNK], F32R,
                            kind="ExternalInput")
    outs = nc.dram_tensor("outs", [128, 1], F32, kind="ExternalOutput")

    with tile.TileContext(nc) as tc:
        with (
            tc.tile_pool(name="singles", bufs=1) as singles,
            tc.tile_pool(name="psall", bufs=1, space="PSUM") as psall,
        ):
            lw_t = singles.tile([128, 128 * ROW_SLOTS], F32R)
            lw_dma = nc.sync.dma_start(out=lw_t[:], in_=lwt[:])

            red = singles.tile([128, 8 * NFILLS], F32)
            tiny_a = singles.tile([128, 1], F32)
            tiny_v = singles.tile([128, 1], F32)
            tiny_v2 = singles.tile([128, 1], F32)

            rhs_all = singles.tile([128, NFILLS, CHUNK], F32R)
            ps_all = psall.tile([128, 2, NGROUPS, NT], F32)

            # Prefill constant rows (norm_col k=2, ones k=3) across all
            # slots, split in two chunks so fill 0 is not gated on the tail.
            rall = rhs_all.rearrange("(g r) s c -> g r (s c)", r=32)
            pres = []
            for lo in range(2):
                cols = slice(8 * CHUNK * lo, 8 * CHUNK * (lo + 1))
                pres.append(nc.sync.dma_start(out=rall[:, 2, cols], in_=ncpre[:]))
                pres.append(nc.sync.dma_start(out=rall[:, 3, cols], in_=onepre[:]))

            act_hist = []
            dve_hist = []
            fill_dmas = []
            last_mm = None
            for f in range(NFILLS):
                rhs = rhs_all[:, f, :]
                rg = rhs.rearrange("(g r) c -> g r c", r=32)

                # x0/x1 rows: partitions {32g+k}, dep-free (fresh slot).
                for k in range(2):
                    dma = nc.sync.dma_start(
                        out=rg[:, k, :],
                        in_=xd[f, k, :, :],
                    )
                    fill_dmas.append(dma)

                def make_carriers():
                    # Narrow per-engine carriers absorbing the last TWO
                    # consumers' in-place WAW ticks (both PSUM parities);
                    # matmuls hang their slot-WAR deps off them so the
                    # carrier -> matmul -> consumer order is structural.
                    ca = cv = None
                    if act_hist:
                        ca = nc.scalar.activation(
                            out=tiny_a[:], in_=red[:, 0:1],
                            func=mybir.ActivationFunctionType.Relu,
                        )
                        for ins in act_hist[-2:]:
                            add_dep_helper(ca.ins, ins, reason="ACT WAW carry")
                    if dve_hist:
                        cv = nc.vector.tensor_scalar(
                            out=tiny_v[:], in0=red[:, 1:2],
                            scalar1=0.0, scalar2=None, op0=mybir.AluOpType.add,
                        )
                        for ins in dve_hist[-2:]:
                            add_dep_helper(cv.ins, ins, reason="DVE WAW carry")
                    return ca, cv

                # Carriers precede the observers: the observer chain rides
                # the ACT carrier, so observers and matmuls share readiness
                # and emission priority puts observers first.
                ca, cv = make_carriers()

                # Observer matmuls: the first absorbs the slot-0 psum cell
                # WAR (merged with the ACT carrier tick; lwt DMA at f=0);
                # the K=1..2 cascade absorbs the x0/x1 DMA lanes; K=3..4 at
                # f in {0,8} also absorb the prefill chunk lanes.
                obs_src = lw_t[0:4, 0:1].bitcast(F32)
                obs = nc.tensor.matmul(
                    ps_all[0:1, 0, 0, 0:1], obs_src, obs_src,
                    start=True, stop=True, tile_position=(0, 0),
                )
                if ca is not None:
                    add_dep_helper(obs.ins, ca.ins, reason="obs after carrier")
                kmax = 4 if f in (0, 8) else 2
                last_obs = None
                for k in range(kmax):
                    cobs = nc.tensor.matmul(
                        ps_all[0:1, 0, 0, 0:1],
                        rhs[0 : k + 1, 0:1].bitcast(F32),
                        rhs[0 : k + 1, 0:1].bitcast(F32),
                        start=True, stop=True, tile_position=(0, 0),
                    )
                    if ca is not None:
                        add_dep_helper(cobs.ins, ca.ins, reason="obs after carrier")
                    last_obs = cobs

                for q in range(4):
                    Q = 4 * f + q
                    p = q % 2
                    s = 2 * f + (q // 2)
                    if q == 2:
                        ca, cv = make_carriers()

                    # One N=512 matmul per group per quarter into parity set
                    # p; the other parity's banks are being drained by the
                    # consumers concurrently (double buffering).
                    for g in range(NGROUPS):
                        col0 = q * NT
                        last_mm = nc.tensor.matmul(
                            ps_all[:, p, g, :],
                            lw_t[32 * g : 32 * g + 4,
                                 128 * s : 128 * (s + 1)],
                            rhs[32 * g : 32 * g + 4, col0 : col0 + NT],
                            start=True, stop=True,
                            tile_position=(32 * g, 0),
                        )
                        carrier = ca if g < 2 else cv
                        if carrier is not None:
                            add_dep_helper(
                                last_mm.ins, carrier.ins,
                                reason="slot WAR via carrier",
                            )
                        add_dep_helper(
                            last_mm.ins, last_obs.ins,
                            reason="mm after observers",
                        )

                    # Consumers: ACT owns groups {0,1}, DVE owns {2,3};
                    # FD=1024 over the parity-p bank set.
                    i_red = 2 * Q
                    cons_a = nc.scalar.activation(
                        out=ps_all[:, p, 0:2, :],
                        in_=ps_all[:, p, 0:2, :],
                        func=mybir.ActivationFunctionType.Relu,
                        accum_out=red[:, i_red : i_red + 1],
                    )
                    act_hist.append(cons_a.ins)
                    cons_v = nc.vector.tensor_scalar(
                        out=ps_all[:, p, 2:4, :],
                        in0=ps_all[:, p, 2:4, :],
                        scalar1=0.0,
                        scalar2=None,
                        op0=mybir.AluOpType.max,
                        op1=mybir.AluOpType.add,
                        accum_out=red[:, i_red + 1 : i_red + 2],
                    )
                    dve_hist.append(cons_v.ins)

            # Final DVE carrier observes ACT's tail so the reduce needs one
            # wait (red col 0 is ACT-written: merges into the same sem).
            cfin = nc.vector.tensor_scalar(
                out=tiny_v2[:], in0=red[:, 0:1],
                scalar1=0.0, scalar2=None, op0=mybir.AluOpType.add,
            )
            for ins in act_hist[-2:]:
                add_dep_helper(cfin.ins, ins, reason="reduce observes ACT tail")
            outs_t = singles.tile([128, 1], F32)
            rsum = nc.vector.reduce_sum(outs_t[:], red[:],
                                        axis=mybir.AxisListType.X)
            # SWDGE: separate lane space -> single wait (the reduce).
            odma = nc.gpsimd.dma_start(out=outs[:], in_=outs_t[:])

            # Pre-observe every proc on SP so the TileContext-exit drain
            # (single-wait NOP) has nothing left to wait on.
            drain_deps = [act_hist[-1], rsum.ins, last_mm.ins, odma.ins,
                          lw_dma.ins]
            drain_deps += [p.ins for p in pres]
            drain_deps += [d.ins for d in fill_dmas[-8:]]
            for dins in drain_deps:
                dr = nc.sync.drain(fusable=False)
                add_dep_helper(dr.ins, dins, reason="pre-drain observe")

    _NC_CACHE = nc
    return nc


def kernel(x, W1, b1, W2, b2, W3, b3):
    global LAST_RESULT
    x = np.asarray(x, dtype=np.float32)
    W1 = np.asarray(W1, dtype=np.float32)
    b1 = np.asarray(b1, dtype=np.float32)
    W2 = np.asarray(W2, dtype=np.float32)
    b2 = np.asarray(b2, dtype=np.float32)
    W3 = np.asarray(W3, dtype=np.float32)
    b3 = np.asarray(b3, dtype=np.float32)

    x0, x1 = x[0], x[1]
    mask = x0 != 0.0

    rows_any = mask.any(axis=1)
    cols_any = mask.any(axis=0)
    ridx = np.nonzero(rows_any)[0]
    cidx = np.nonzero(cols_any)[0]
    rmin, rmax = float(ridx[0]), float(ridx[-1])
    cmin, cmax = float(cidx[0]), float(cidx[-1])

    W12 = W1.astype(np.float64) @ W2.astype(np.float64)
    b12 = b1.astype(np.float64) @ W2.astype(np.float64) + b2
    v0 = W12[0]

    nr_all = (np.arange(H, dtype=np.float64) - rmin) / (rmax - rmin)
    nc_all = (np.arange(W, dtype=np.float64) - cmin) / (cmax - cmin)

    nc2 = np.tile(nc_all.astype(np.float32), 2)
    v2f = W12[2].astype(np.float32)
    v3f = W12[3].astype(np.float32)
    v1f = W12[1].astype(np.float32)

    ncpre = np.broadcast_to(
        np.tile(nc2, 8)[None, :], (NGROUPS, 8 * CHUNK)
    ).copy()
    onepre = np.ones((NGROUPS, 8 * CHUNK), dtype=np.float32)

    nc_prog = _build_bass()
    in_maps = []
    for c in range(N_CORES):
        shard = x[:, c * ROWS_PER_CORE : (c + 1) * ROWS_PER_CORE, :]
        xdv = np.empty((NFILLS, 2, NGROUPS, CHUNK), dtype=np.float32)
        sh = shard.reshape(2, NFILLS, NGROUPS, CHUNK)
        xdv[:, 0, :, :] = sh[0]
        xdv[:, 1, :, :] = sh[1]

        btab = (
            b12[:, None]
            + np.outer(v0, nr_all[c * ROWS_PER_CORE : (c + 1) * ROWS_PER_CORE])
        ).astype(np.float32)
        lwtv = np.zeros((128, 128 * ROW_SLOTS), dtype=np.float32)
        for g in range(NGROUPS):
            for s in range(ROW_SLOTS):
                f, half = divmod(s, 2)
                r_loc = 8 * f + 2 * g + half
                blk = slice(128 * s, 128 * (s + 1))
                lwtv[32 * g + 0, blk] = v2f
                lwtv[32 * g + 1, blk] = v3f
                lwtv[32 * g + 2, blk] = v1f
                lwtv[32 * g + 3, blk] = btab[:, r_loc]
        in_maps.append({"xd": xdv, "lwt": lwtv, "ncpre": ncpre,
                        "onepre": onepre})

    res = run_bass_kernel_spmd(
        nc_prog, in_maps, core_ids=list(range(N_CORES)), trace=TRACE
    )
    LAST_RESULT = res

    S = np.zeros(D, dtype=np.float64)
    for c in range(N_CORES):
        S += res.results[c]["outs"][:, 0].astype(np.float64)

    if not mask.all():
        zr, zc = np.nonzero(~mask)
        hz = (
            np.outer(nr_all[zr], W12[0])
            + np.outer(nc_all[zc], W12[1])
            + np.outer(x1[zr, zc].astype(np.float64), W12[3])
            + b12[None, :]
        )
        S -= np.maximum(hz, 0.0).sum(axis=0)

    wsum = float(mask.sum())
    out = (S @ W3.astype(np.float64)) / wsum + b3.astype(np.float64)
    return out.astype(np.float32)


# revision 34
# speedup vs baseline: 1.2023x; 1.0206x over previous
"""Trainium2 Bass kernel for nn_Encoder_74182675137046.

Reference computation (per image of 1024x1024 complex pixels):
    feats = [norm_row, norm_col, x0, x1]  per pixel     [N, 4]
    h   = relu((feats @ W1 + b1) @ W2 + b2)             [N, 128]
    out = h @ W3 + b3                                   [N, 128]
    result = (w * out).sum(0) / w.sum()                 [128]
with w = (x0 != 0), and norm_row/col normalized by masked min/max.

Algebraic folding (exact):
    fc1+fc2 fold:  h_pre = feats @ W12 + b12,  W12 = W1@W2, b12 = b1@W2 + b2
    pool/fc3 swap: (w*out).sum = (sum_p w_p*relu(h_pre_p)) @ W3 + w.sum()*b3
So the device only computes S = sum_p relu(h_pre_p)  (a [128] vector per
core); the tiny [128]x[128,128] tail runs on host in float64.

Device design (per core, 128 image rows = 131072 points): see the build
function. Key points: four tile_position row groups with K=4 feature rows
(x0, x1, norm_col, ones) at partitions {32g+k}; float32r single-pass
matmuls with the per-image-row bias folded into lhsT row 3; consumers
read adjacent PSUM slot pairs as FD=2048 relu+accумulate ops split between
ScalarE and VectorE; per-fill rhs slots (no reuse -> dep-free DMAs) with
prefilled constant rows; observer matmuls / narrow carriers keep every
instruction within the single TPB sync-wait slot.
"""

import numpy as np

import concourse.bass as bass
import concourse.tile as tile
from concourse import mybir
from concourse.bass_utils import run_bass_kernel_spmd
from concourse.tile_rust import add_dep_helper

H = 1024
W = 1024
D = 128
N_CORES = 8
ROWS_PER_CORE = H // N_CORES          # 128
NPTS = ROWS_PER_CORE * W              # 131072
CHUNK = 2 * W                         # 2048 pts per (group, fill) = 2 image rows
NGROUPS = 4
FILL_PTS = NGROUPS * CHUNK            # 8192
NFILLS = NPTS // FILL_PTS             # 16
NT = 512
ROW_SLOTS = 2 * NFILLS                # lhsT blocks per group (32)

F32 = mybir.dt.float32
F32R = mybir.dt.float32r

TRACE = False
LAST_RESULT = None

_NC_CACHE = None


def _build_bass():
    """Build the SPMD Bass program (same program on all 8 cores)."""
    global _NC_CACHE
    if _NC_CACHE is not None:
        return _NC_CACHE

    nc = bass.Bass()

    xd = nc.dram_tensor("xd", [NFILLS, 2, NGROUPS, CHUNK], F32R,
                        kind="ExternalInput")
    lwt = nc.dram_tensor("lwt", [128, 128 * ROW_SLOTS], F32R,
                         kind="ExternalInput")
    ncpre = nc.dram_tensor("ncpre", [NGROUPS, 2 * CHUNK], F32R,
                           kind="ExternalInput")
    onepre = nc.dram_tensor("onepre", [NGROUPS, 2 * CHUNK], F32R,
                            kind="ExternalInput")
    outs = nc.dram_tensor("outs", [128, 1], F32, kind="ExternalOutput")

    with tile.TileContext(nc) as tc:
        with (
            tc.tile_pool(name="singles", bufs=1) as singles,
            tc.tile_pool(name="psall", bufs=1, space="PSUM") as psall,
        ):
            lw_t = singles.tile([128, 128 * ROW_SLOTS], F32R)
            lw_dma = nc.sync.dma_start(out=lw_t[:], in_=lwt[:])

            red = singles.tile([128, 8 * NFILLS], F32)
            tiny_a = singles.tile([128, 1], F32)
            tiny_v = singles.tile([128, 1], F32)
            tiny_v2 = singles.tile([128, 1], F32)

            rhs_all = singles.tile([128, NFILLS, CHUNK], F32R)
            ps_all = psall.tile([128, 2, NGROUPS, NT], F32)

            # Prefill constant rows (norm_col k=2, ones k=3) across all
            # slots, split in two chunks so fill 0 is not gated on the tail.
            rall = rhs_all.rearrange("(g r) s c -> g r (s c)", r=32)
            pres = []
            for lo in range(8):
                cols = slice(2 * CHUNK * lo, 2 * CHUNK * (lo + 1))
                pres.append(nc.sync.dma_start(out=rall[:, 2, cols], in_=ncpre[:]))
                pres.append(nc.sync.dma_start(out=rall[:, 3, cols], in_=onepre[:]))

            act_hist = []
            dve_hist = []
            fill_dmas = []
            last_mm = None
            for f in range(NFILLS):
                rhs = rhs_all[:, f, :]
                rg = rhs.rearrange("(g r) c -> g r c", r=32)

                # x0/x1 rows: partitions {32g+k}, dep-free (fresh slot).
                for k in range(2):
                    dma = nc.sync.dma_start(
                        out=rg[:, k, :],
                        in_=xd[f, k, :, :],
                    )
                    fill_dmas.append(dma)

                def make_carriers():
                    # Narrow per-engine carriers absorbing the last TWO
                    # consumers' in-place WAW ticks (both PSUM parities);
                    # matmuls hang their slot-WAR deps off them so the
                    # carrier -> matmul -> consumer order is structural.
                    ca = cv = None
                    if len(act_hist) >= 2:
                        ca = nc.scalar.activation(
                            out=tiny_a[:], in_=red[:, 0:1],
                            func=mybir.ActivationFunctionType.Relu,
                        )
                        add_dep_helper(ca.ins, act_hist[-2], reason="ACT WAW")
                    if len(dve_hist) >= 2:
                        cv = nc.vector.tensor_scalar(
                            out=tiny_v[:], in0=red[:, 1:2],
                            scalar1=0.0, scalar2=None, op0=mybir.AluOpType.add,
                        )
                        add_dep_helper(cv.ins, dve_hist[-2], reason="DVE WAW")
                    return ca, cv

                # Carriers precede the observers: the observer chain rides
                # the ACT carrier, so observers and matmuls share readiness
                # and emission priority puts observers first.
                ca, cv = make_carriers()

                # Observer matmuls: the first absorbs the slot-0 psum cell
                # WAR (merged with the ACT carrier tick; lwt DMA at f=0);
                # the K=1..2 cascade absorbs the x0/x1 DMA lanes; K=3..4 at
                # f in {0,8} also absorb the prefill chunk lanes.
                obs_src = lw_t[0:4, 0:1].bitcast(F32)
                obs = nc.tensor.matmul(
                    ps_all[0:1, 0, 0, 0:1], obs_src, obs_src,
                    start=True, stop=True, tile_position=(0, 0),
                )
                if ca is not None:
                    add_dep_helper(obs.ins, ca.ins, reason="obs after carrier")
                kmax = 4 if f in (0, 8) else 2
                last_obs = None
                for k in range(kmax):
                    cobs = nc.tensor.matmul(
                        ps_all[0:1, 0, 0, 0:1],
                        rhs[0 : k + 1, 0:1].bitcast(F32),
                        rhs[0 : k + 1, 0:1].bitcast(F32),
                        start=True, stop=True, tile_position=(0, 0),
                    )
                    if ca is not None:
                        add_dep_helper(cobs.ins, ca.ins, reason="obs after carrier")
                    last_obs = cobs

                for q in range(4):
                    Q = 4 * f + q
                    p = q % 2
                    s = 2 * f + (q // 2)
                    if Q > 0:
                        ca, cv = make_carriers()

                    # One N=512 matmul per group per quarter into parity set
                    # p; the other parity's banks are being drained by the
                    # consumers concurrently (double buffering).
                    for g in range(NGROUPS):
                        col0 = q * NT
                        last_mm = nc.tensor.matmul(
                            ps_all[:, p, g, :],
                            lw_t[32 * g : 32 * g + 4,
                                 128 * s : 128 * (s + 1)],
                            rhs[32 * g : 32 * g + 4, col0 : col0 + NT],
                            start=True, stop=True,
                            tile_position=(32 * g, 0),
                        )
                        carrier = ca if g < 2 else cv
                        if carrier is not None:
                            add_dep_helper(
                                last_mm.ins, carrier.ins,
                                reason="slot WAR via carrier",
                            )
                        add_dep_helper(
                            last_mm.ins, last_obs.ins,
                            reason="mm after observers",
                        )

                    # Consumers: ACT owns groups {0,1}, DVE owns {2,3};
                    # FD=1024 over the parity-p bank set.
                    i_red = 2 * Q
                    cons_a = nc.scalar.activation(
                        out=ps_all[:, p, 0:2, :],
                        in_=ps_all[:, p, 0:2, :],
                        func=mybir.ActivationFunctionType.Relu,
                        accum_out=red[:, i_red : i_red + 1],
                    )
                    act_hist.append(cons_a.ins)
                    cons_v = nc.vector.tensor_scalar(
                        out=ps_all[:, p, 2:4, :],
                        in0=ps_all[:, p, 2:4, :],
                        scalar1=0.0,
                        scalar2=None,
                        op0=mybir.AluOpType.max,
                        op1=mybir.AluOpType.add,
                        accum_out=red[:, i_red + 1 : i_red + 2],
                    )
                    dve_hist.append(cons_v.ins)

            # Final DVE carrier observes ACT's tail so the reduce needs one
            # wait (red col 0 is ACT-written: merges into the same sem).
            cfin = nc.vector.tensor_scalar(
                out=tiny_v2[:], in0=red[:, 0:1],
                scalar1=0.0, scalar2=None, op0=mybir.AluOpType.add,
            )
            for ins in act_hist[-2:]:
                add_dep_helper(cfin.ins, ins, reason="reduce observes ACT tail")
            outs_t = singles.tile([128, 1], F32)
            rsum = nc.vector.reduce_sum(outs_t[:], red[:],
                                        axis=mybir.AxisListType.X)
            # SWDGE: separate lane space -> single wait (the reduce).
            odma = nc.gpsimd.dma_start(out=outs[:], in_=outs_t[:])

            # Pre-observe every proc on SP so the TileContext-exit drain
            # (single-wait NOP) has nothing left to wait on.
            drain_deps = [act_hist[-1], rsum.ins, last_mm.ins, odma.ins,
                          lw_dma.ins]
            drain_deps += [p.ins for p in pres]
            drain_deps += [d.ins for d in fill_dmas[-8:]]
            for dins in drain_deps:
                dr = nc.sync.drain(fusable=False)
                add_dep_helper(dr.ins, dins, reason="pre-drain observe")

    _NC_CACHE = nc
    return nc


def kernel(x, W1, b1, W2, b2, W3, b3):
    global LAST_RESULT
    x = np.asarray(x, dtype=np.float32)
    W1 = np.asarray(W1, dtype=np.float32)
    b1 = np.asarray(b1, dtype=np.float32)
    W2 = np.asarray(W2, dtype=np.float32)
    b2 = np.asarray(b2, dtype=np.float32)
    W3 = np.asarray(W3, dtype=np.float32)
    b3 = np.asarray(b3, dtype=np.float32)

    x0, x1 = x[0], x[1]
    mask = x0 != 0.0

    rows_any = mask.any(axis=1)
    cols_any = mask.any(axis=0)
    ridx = np.nonzero(rows_any)[0]
    cidx = np.nonzero(cols_any)[0]
    rmin, rmax = float(ridx[0]), float(ridx[-1])
    cmin, cmax = float(cidx[0]), float(cidx[-1])

    W12 = W1.astype(np.float64) @ W2.astype(np.float64)
    b12 = b1.astype(np.float64) @ W2.astype(np.float64) + b2
    v0 = W12[0]

    nr_all = (np.arange(H, dtype=np.float64) - rmin) / (rmax - rmin)
    nc_all = (np.arange(W, dtype=np.float64) - cmin) / (cmax - cmin)

    nc2 = np.tile(nc_all.astype(np.float32), 2)
    v2f = W12[2].astype(np.float32)
    v3f = W12[3].astype(np.float32)
    v1f = W12[1].astype(np.float32)

    ncpre = np.broadcast_to(
        np.tile(nc2, 2)[None, :], (NGROUPS, 2 * CHUNK)
    ).copy()
    onepre = np.ones((NGROUPS, 2 * CHUNK), dtype=np.float32)

    nc_prog = _build_bass()
    in_maps = []
    for c in range(N_CORES):
        shard = x[:, c * ROWS_PER_CORE : (c + 1) * ROWS_PER_CORE, :]
        xdv = np.empty((NFILLS, 2, NGROUPS, CHUNK), dtype=np.float32)
        sh = shard.reshape(2, NFILLS, NGROUPS, CHUNK)
        xdv[:, 0, :, :] = sh[0]
        xdv[:, 1, :, :] = sh[1]

        btab = (
            b12[:, None]
            + np.outer(v0, nr_all[c * ROWS_PER_CORE : (c + 1) * ROWS_PER_CORE])
        ).astype(np.float32)
        lwtv = np.zeros((128, 128 * ROW_SLOTS), dtype=np.float32)
        for g in range(NGROUPS):
            for s in range(ROW_SLOTS):
                f, half = divmod(s, 2)
                r_loc = 8 * f + 2 * g + half
                blk = slice(128 * s, 128 * (s + 1))
                lwtv[32 * g + 0, blk] = v2f
                lwtv[32 * g + 1, blk] = v3f
                lwtv[32 * g + 2, blk] = v1f
                lwtv[32 * g + 3, blk] = btab[:, r_loc]
        in_maps.append({"xd": xdv, "lwt": lwtv, "ncpre": ncpre,
                        "onepre": onepre})

    res = run_bass_kernel_spmd(
        nc_prog, in_maps, core_ids=list(range(N_CORES)), trace=TRACE
    )
    LAST_RESULT = res

    S = np.zeros(D, dtype=np.float64)
    for c in range(N_CORES):
        S += res.results[c]["outs"][:, 0].astype(np.float64)

    if not mask.all():
        zr, zc = np.nonzero(~mask)
        hz = (
            np.outer(nr_all[zr], W12[0])
            + np.outer(nc_all[zc], W12[1])
            + np.outer(x1[zr, zc].astype(np.float64), W12[3])
            + b12[None, :]
        )
        S -= np.maximum(hz, 0.0).sum(axis=0)

    wsum = float(mask.sum())
    out = (S @ W3.astype(np.float64)) / wsum + b3.astype(np.float64)
    return out.astype(np.float32)


# revision 35
# speedup vs baseline: 1.3345x; 1.1099x over previous
"""Trainium2 Bass kernel for nn_Encoder_74182675137046.

Reference computation (per image of 1024x1024 complex pixels):
    feats = [norm_row, norm_col, x0, x1]  per pixel     [N, 4]
    h   = relu((feats @ W1 + b1) @ W2 + b2)             [N, 128]
    out = h @ W3 + b3                                   [N, 128]
    result = (w * out).sum(0) / w.sum()                 [128]
with w = (x0 != 0), and norm_row/col normalized by masked min/max.

Algebraic folding (exact):
    fc1+fc2 fold:  h_pre = feats @ W12 + b12,  W12 = W1@W2, b12 = b1@W2 + b2
    pool/fc3 swap: (w*out).sum = (sum_p w_p*relu(h_pre_p)) @ W3 + w.sum()*b3
So the device only computes S = sum_p relu(h_pre_p)  (a [128] vector per
core); the tiny [128]x[128,128] tail runs on host in float64.

Device design (per core, 128 image rows = 131072 points): see the build
function. Key points: four tile_position row groups with K=4 feature rows
(x0, x1, norm_col, ones) at partitions {32g+k}; float32r single-pass
matmuls with the per-image-row bias folded into lhsT row 3; consumers
read adjacent PSUM slot pairs as FD=2048 relu+accумulate ops split between
ScalarE and VectorE; per-fill rhs slots (no reuse -> dep-free DMAs) with
prefilled constant rows; observer matmuls / narrow carriers keep every
instruction within the single TPB sync-wait slot.
"""

import numpy as np

import concourse.bass as bass
import concourse.tile as tile
from concourse import mybir
from concourse.bass_utils import run_bass_kernel_spmd
from concourse.tile_rust import add_dep_helper

H = 1024
W = 1024
D = 128
N_CORES = 8
ROWS_PER_CORE = H // N_CORES          # 128
NPTS = ROWS_PER_CORE * W              # 131072
CHUNK = 2 * W                         # 2048 pts per (group, fill) = 2 image rows
NGROUPS = 4
FILL_PTS = NGROUPS * CHUNK            # 8192
NFILLS = NPTS // FILL_PTS             # 16
NT = 512
ROW_SLOTS = 2 * NFILLS                # lhsT blocks per group (32)

F32 = mybir.dt.float32
F32R = mybir.dt.float32r

TRACE = False
LAST_RESULT = None

_NC_CACHE = None


def _build_bass():
    """Build the SPMD Bass program (same program on all 8 cores)."""
    global _NC_CACHE
    if _NC_CACHE is not None:
        return _NC_CACHE

    nc = bass.Bass()

    xd = nc.dram_tensor("xd", [NFILLS, 2, NGROUPS, CHUNK], F32R,
                        kind="ExternalInput")
    lwt = nc.dram_tensor("lwt", [128, 128 * ROW_SLOTS], F32R,
                         kind="ExternalInput")
    ncpre = nc.dram_tensor("ncpre", [NGROUPS, 2 * CHUNK], F32R,
                           kind="ExternalInput")
    onepre = nc.dram_tensor("onepre", [NGROUPS, 2 * CHUNK], F32R,
                            kind="ExternalInput")
    outs = nc.dram_tensor("outs", [128, 1], F32, kind="ExternalOutput")

    with tile.TileContext(nc) as tc:
        with (
            tc.tile_pool(name="singles", bufs=1) as singles,
            tc.tile_pool(name="psall", bufs=1, space="PSUM") as psall,
        ):
            lw_t = singles.tile([128, 128 * ROW_SLOTS], F32R)
            lw_dma = nc.gpsimd.dma_start(out=lw_t[:], in_=lwt[:])

            red = singles.tile([128, 8 * NFILLS], F32)
            tiny_a = singles.tile([128, 1], F32)
            tiny_v = singles.tile([128, 1], F32)
            tiny_v2 = singles.tile([128, 1], F32)

            rhs_all = singles.tile([128, NFILLS, CHUNK], F32R)
            ps_all = psall.tile([128, 2, NGROUPS, NT], F32)

            # Prefill constant rows (norm_col k=2, ones k=3) across all
            # slots, split in two chunks so fill 0 is not gated on the tail.
            rall = rhs_all.rearrange("(g r) s c -> g r (s c)", r=32)
            pres = []

            act_hist = []
            dve_hist = []
            fill_dmas = []
            last_mm = None
            for f in range(NFILLS):
                rhs = rhs_all[:, f, :]
                rg = rhs.rearrange("(g r) c -> g r c", r=32)

                if f % 2 == 0:
                    cols = slice(CHUNK * f, CHUNK * (f + 2))
                    pres.append(
                        nc.sync.dma_start(out=rall[:, 2, cols], in_=ncpre[:])
                    )
                    pres.append(
                        nc.sync.dma_start(out=rall[:, 3, cols], in_=onepre[:])
                    )

                # x0/x1 rows: partitions {32g+k}, dep-free (fresh slot).
                for k in range(2):
                    dma = nc.sync.dma_start(
                        out=rg[:, k, :],
                        in_=xd[f, k, :, :],
                    )
                    fill_dmas.append(dma)

                def make_carriers():
                    # Narrow per-engine carriers absorbing the last TWO
                    # consumers' in-place WAW ticks (both PSUM parities);
                    # matmuls hang their slot-WAR deps off them so the
                    # carrier -> matmul -> consumer order is structural.
                    ca = cv = None
                    if len(act_hist) >= 2:
                        ca = nc.scalar.activation(
                            out=tiny_a[:], in_=red[:, 0:1],
                            func=mybir.ActivationFunctionType.Relu,
                        )
                        add_dep_helper(ca.ins, act_hist[-2], reason="ACT WAW")
                    if len(dve_hist) >= 2:
                        cv = nc.vector.tensor_scalar(
                            out=tiny_v[:], in0=red[:, 1:2],
                            scalar1=0.0, scalar2=None, op0=mybir.AluOpType.add,
                        )
                        add_dep_helper(cv.ins, dve_hist[-2], reason="DVE WAW")
                    return ca, cv

                # Carriers precede the observers: the observer chain rides
                # the ACT carrier, so observers and matmuls share readiness
                # and emission priority puts observers first.
                ca, cv = make_carriers()

                # Observer matmuls: the first absorbs the slot-0 psum cell
                # WAR (merged with the ACT carrier tick; lwt DMA at f=0);
                # the K=1..2 cascade absorbs the x0/x1 DMA lanes; K=3..4 at
                # f in {0,8} also absorb the prefill chunk lanes.
                obs_src = lw_t[0:4, 0:1].bitcast(F32)
                obs = nc.tensor.matmul(
                    ps_all[0:1, 0, 0, 0:1], obs_src, obs_src,
                    start=True, stop=True, tile_position=(0, 0),
                )
                if ca is not None:
                    add_dep_helper(obs.ins, ca.ins, reason="obs after carrier")
                kmax = 4 if f % 2 == 0 else 2
                last_obs = None
                for k in range(kmax):
                    cobs = nc.tensor.matmul(
                        ps_all[0:1, 0, 0, 0:1],
                        rhs[0 : k + 1, 0:1].bitcast(F32),
                        rhs[0 : k + 1, 0:1].bitcast(F32),
                        start=True, stop=True, tile_position=(0, 0),
                    )
                    if ca is not None:
                        add_dep_helper(cobs.ins, ca.ins, reason="obs after carrier")
                    last_obs = cobs

                for q in range(4):
                    Q = 4 * f + q
                    p = q % 2
                    s = 2 * f + (q // 2)
                    if Q > 0:
                        ca, cv = make_carriers()

                    # One N=512 matmul per group per quarter into parity set
                    # p; the other parity's banks are being drained by the
                    # consumers concurrently (double buffering).
                    for g in range(NGROUPS):
                        col0 = q * NT
                        last_mm = nc.tensor.matmul(
                            ps_all[:, p, g, :],
                            lw_t[32 * g : 32 * g + 4,
                                 128 * s : 128 * (s + 1)],
                            rhs[32 * g : 32 * g + 4, col0 : col0 + NT],
                            start=True, stop=True,
                            tile_position=(32 * g, 0),
                        )
                        carrier = ca if g < 2 else cv
                        if carrier is not None:
                            add_dep_helper(
                                last_mm.ins, carrier.ins,
                                reason="slot WAR via carrier",
                            )
                        add_dep_helper(
                            last_mm.ins, last_obs.ins,
                            reason="mm after observers",
                        )

                    # Consumers: ACT owns groups {0,1}, DVE owns {2,3};
                    # FD=1024 over the parity-p bank set.
                    i_red = 2 * Q
                    cons_a = nc.scalar.activation(
                        out=ps_all[:, p, 0:2, :],
                        in_=ps_all[:, p, 0:2, :],
                        func=mybir.ActivationFunctionType.Relu,
                        accum_out=red[:, i_red : i_red + 1],
                    )
                    act_hist.append(cons_a.ins)
                    cons_v = nc.vector.tensor_scalar(
                        out=ps_all[:, p, 2:4, :],
                        in0=ps_all[:, p, 2:4, :],
                        scalar1=0.0,
                        scalar2=None,
                        op0=mybir.AluOpType.max,
                        op1=mybir.AluOpType.add,
                        accum_out=red[:, i_red + 1 : i_red + 2],
                    )
                    dve_hist.append(cons_v.ins)

            # Final DVE carrier observes ACT's tail so the reduce needs one
            # wait (red col 0 is ACT-written: merges into the same sem).
            cfin = nc.vector.tensor_scalar(
                out=tiny_v2[:], in0=red[:, 0:1],
                scalar1=0.0, scalar2=None, op0=mybir.AluOpType.add,
            )
            for ins in act_hist[-2:]:
                add_dep_helper(cfin.ins, ins, reason="reduce observes ACT tail")
            outs_t = singles.tile([128, 1], F32)
            rsum = nc.vector.reduce_sum(outs_t[:], red[:],
                                        axis=mybir.AxisListType.X)
            # SWDGE: separate lane space -> single wait (the reduce).
            odma = nc.gpsimd.dma_start(out=outs[:], in_=outs_t[:])

            # Pre-observe every proc on SP so the TileContext-exit drain
            # (single-wait NOP) has nothing left to wait on.
            drain_deps = [act_hist[-1], rsum.ins, last_mm.ins, odma.ins,
                          lw_dma.ins]
            drain_deps += [p.ins for p in pres]
            drain_deps += [d.ins for d in fill_dmas[-8:]]
            for dins in drain_deps:
                dr = nc.sync.drain(fusable=False)
                add_dep_helper(dr.ins, dins, reason="pre-drain observe")

    _NC_CACHE = nc
    return nc


def kernel(x, W1, b1, W2, b2, W3, b3):
    global LAST_RESULT
    x = np.asarray(x, dtype=np.float32)
    W1 = np.asarray(W1, dtype=np.float32)
    b1 = np.asarray(b1, dtype=np.float32)
    W2 = np.asarray(W2, dtype=np.float32)
    b2 = np.asarray(b2, dtype=np.float32)
    W3 = np.asarray(W3, dtype=np.float32)
    b3 = np.asarray(b3, dtype=np.float32)

    x0, x1 = x[0], x[1]
    mask = x0 != 0.0

    rows_any = mask.any(axis=1)
    cols_any = mask.any(axis=0)
    ridx = np.nonzero(rows_any)[0]
    cidx = np.nonzero(cols_any)[0]
    rmin, rmax = float(ridx[0]), float(ridx[-1])
    cmin, cmax = float(cidx[0]), float(cidx[-1])

    W12 = W1.astype(np.float64) @ W2.astype(np.float64)
    b12 = b1.astype(np.float64) @ W2.astype(np.float64) + b2
    v0 = W12[0]

    nr_all = (np.arange(H, dtype=np.float64) - rmin) / (rmax - rmin)
    nc_all = (np.arange(W, dtype=np.float64) - cmin) / (cmax - cmin)

    nc2 = np.tile(nc_all.astype(np.float32), 2)
    v2f = W12[2].astype(np.float32)
    v3f = W12[3].astype(np.float32)
    v1f = W12[1].astype(np.float32)

    ncpre = np.broadcast_to(
        np.tile(nc2, 2)[None, :], (NGROUPS, 2 * CHUNK)
    ).copy()
    onepre = np.ones((NGROUPS, 2 * CHUNK), dtype=np.float32)

    nc_prog = _build_bass()
    in_maps = []
    for c in range(N_CORES):
        shard = x[:, c * ROWS_PER_CORE : (c + 1) * ROWS_PER_CORE, :]
        xdv = np.empty((NFILLS, 2, NGROUPS, CHUNK), dtype=np.float32)
        sh = shard.reshape(2, NFILLS, NGROUPS, CHUNK)
        xdv[:, 0, :, :] = sh[0]
        xdv[:, 1, :, :] = sh[1]

        btab = (
            b12[:, None]
            + np.outer(v0, nr_all[c * ROWS_PER_CORE : (c + 1) * ROWS_PER_CORE])
        ).astype(np.float32)
        lwtv = np.zeros((128, 128 * ROW_SLOTS), dtype=np.float32)
        for g in range(NGROUPS):
            for s in range(ROW_SLOTS):
                f, half = divmod(s, 2)
                r_loc = 8 * f + 2 * g + half
                blk = slice(128 * s, 128 * (s + 1))
                lwtv[32 * g + 0, blk] = v2f
                lwtv[32 * g + 1, blk] = v3f
                lwtv[32 * g + 2, blk] = v1f
                lwtv[32 * g + 3, blk] = btab[:, r_loc]
        in_maps.append({"xd": xdv, "lwt": lwtv, "ncpre": ncpre,
                        "onepre": onepre})

    res = run_bass_kernel_spmd(
        nc_prog, in_maps, core_ids=list(range(N_CORES)), trace=TRACE
    )
    LAST_RESULT = res

    S = np.zeros(D, dtype=np.float64)
    for c in range(N_CORES):
        S += res.results[c]["outs"][:, 0].astype(np.float64)

    if not mask.all():
        zr, zc = np.nonzero(~mask)
        hz = (
            np.outer(nr_all[zr], W12[0])
            + np.outer(nc_all[zc], W12[1])
            + np.outer(x1[zr, zc].astype(np.float64), W12[3])
            + b12[None, :]
        )
        S -= np.maximum(hz, 0.0).sum(axis=0)

    wsum = float(mask.sum())
    out = (S @ W3.astype(np.float64)) / wsum + b3.astype(np.float64)
    return out.astype(np.float32)


# revision 37
# speedup vs baseline: 1.3455x; 1.0082x over previous
"""Trainium2 Bass kernel for nn_Encoder_74182675137046.

Reference computation (per image of 1024x1024 complex pixels):
    feats = [norm_row, norm_col, x0, x1]  per pixel     [N, 4]
    h   = relu((feats @ W1 + b1) @ W2 + b2)             [N, 128]
    out = h @ W3 + b3                                   [N, 128]
    result = (w * out).sum(0) / w.sum()                 [128]
with w = (x0 != 0), and norm_row/col normalized by masked min/max.

Algebraic folding (exact):
    fc1+fc2 fold:  h_pre = feats @ W12 + b12,  W12 = W1@W2, b12 = b1@W2 + b2
    pool/fc3 swap: (w*out).sum = (sum_p w_p*relu(h_pre_p)) @ W3 + w.sum()*b3
So the device only computes S = sum_p relu(h_pre_p)  (a [128] vector per
core); the tiny [128]x[128,128] tail runs on host in float64.

Device design (per core, 128 image rows = 131072 points): see the build
function. Key points: four tile_position row groups with K=4 feature rows
(x0, x1, norm_col, ones) at partitions {32g+k}; float32r single-pass
matmuls with the per-image-row bias folded into lhsT row 3; consumers
read adjacent PSUM slot pairs as FD=2048 relu+accумulate ops split between
ScalarE and VectorE; per-fill rhs slots (no reuse -> dep-free DMAs) with
prefilled constant rows; observer matmuls / narrow carriers keep every
instruction within the single TPB sync-wait slot.
"""

import numpy as np

import concourse.bass as bass
import concourse.tile as tile
from concourse import mybir
from concourse.bass_utils import run_bass_kernel_spmd
from concourse.tile_rust import add_dep_helper

H = 1024
W = 1024
D = 128
N_CORES = 8
ROWS_PER_CORE = H // N_CORES          # 128
NPTS = ROWS_PER_CORE * W              # 131072
CHUNK = 2 * W                         # 2048 pts per (group, fill) = 2 image rows
NGROUPS = 4
FILL_PTS = NGROUPS * CHUNK            # 8192
NFILLS = NPTS // FILL_PTS             # 16
NT = 512
ROW_SLOTS = 2 * NFILLS                # lhsT blocks per group (32)

F32 = mybir.dt.float32
F32R = mybir.dt.float32r

TRACE = False
LAST_RESULT = None

_NC_CACHE = None


def _build_bass():
    """Build the SPMD Bass program (same program on all 8 cores)."""
    global _NC_CACHE
    if _NC_CACHE is not None:
        return _NC_CACHE

    nc = bass.Bass()

    xd = nc.dram_tensor("xd", [NFILLS, 2, NGROUPS, CHUNK], F32R,
                        kind="ExternalInput")
    lwt = nc.dram_tensor("lwt", [128, 128 * ROW_SLOTS], F32R,
                         kind="ExternalInput")
    ncpre = nc.dram_tensor("ncpre", [NGROUPS, 2 * CHUNK], F32R,
                           kind="ExternalInput")
    onepre = nc.dram_tensor("onepre", [NGROUPS, 2 * CHUNK], F32R,
                            kind="ExternalInput")
    outs = nc.dram_tensor("outs", [128, 1], F32, kind="ExternalOutput")

    with tile.TileContext(nc) as tc:
        with (
            tc.tile_pool(name="singles", bufs=1) as singles,
            tc.tile_pool(name="psall", bufs=1, space="PSUM") as psall,
        ):
            lw_t = singles.tile([128, 128 * ROW_SLOTS], F32R)
            lw_dma = nc.gpsimd.dma_start(out=lw_t[:], in_=lwt[:])

            red = singles.tile([128, 8 * NFILLS], F32)
            tiny_a = singles.tile([128, 1], F32)
            tiny_v = singles.tile([128, 1], F32)
            tiny_v2 = singles.tile([128, 1], F32)

            rhs_all = singles.tile([128, NFILLS, CHUNK], F32R)
            ps_all = psall.tile([128, 2, NGROUPS, NT], F32)

            # Prefill constant rows (norm_col k=2, ones k=3) across all
            # slots, split in two chunks so fill 0 is not gated on the tail.
            rall = rhs_all.rearrange("(g r) s c -> g r (s c)", r=32)
            pres = []

            act_hist = []
            dve_hist = []
            fill_dmas = []
            last_mm = None
            for f in range(NFILLS):
                rhs = rhs_all[:, f, :]
                rg = rhs.rearrange("(g r) c -> g r c", r=32)

                if f % 2 == 0:
                    cols = slice(CHUNK * f, CHUNK * (f + 2))
                    pres.append(
                        nc.sync.dma_start(out=rall[:, 2, cols], in_=ncpre[:])
                    )
                    pres.append(
                        nc.sync.dma_start(out=rall[:, 3, cols], in_=onepre[:])
                    )

                # x0/x1 rows: partitions {32g+k}, dep-free (fresh slot).
                for k in range(2):
                    dma = nc.sync.dma_start(
                        out=rg[:, k, :],
                        in_=xd[f, k, :, :],
                    )
                    fill_dmas.append(dma)

                def make_carriers():
                    # Narrow per-engine carriers absorbing the last TWO
                    # consumers' in-place WAW ticks (both PSUM parities);
                    # matmuls hang their slot-WAR deps off them so the
                    # carrier -> matmul -> consumer order is structural.
                    ca = cv = None
                    if len(act_hist) >= 2:
                        ca = nc.scalar.activation(
                            out=tiny_a[:], in_=red[:, 0:1],
                            func=mybir.ActivationFunctionType.Relu,
                        )
                        add_dep_helper(ca.ins, act_hist[-2], reason="ACT WAW")
                    if len(dve_hist) >= 2:
                        cv = nc.vector.tensor_scalar(
                            out=tiny_v[:], in0=red[:, 1:2],
                            scalar1=0.0, scalar2=None, op0=mybir.AluOpType.add,
                        )
                        add_dep_helper(cv.ins, dve_hist[-2], reason="DVE WAW")
                    return ca, cv

                # Carriers precede the observers: the observer chain rides
                # the ACT carrier, so observers and matmuls share readiness
                # and emission priority puts observers first.
                ca, cv = make_carriers()

                # Observer matmuls: the first absorbs the slot-0 psum cell
                # WAR (merged with the ACT carrier tick; lwt DMA at f=0);
                # the K=1..2 cascade absorbs the x0/x1 DMA lanes; K=3..4 at
                # f in {0,8} also absorb the prefill chunk lanes.
                obs_src = lw_t[0:4, 0:1].bitcast(F32)
                obs = nc.tensor.matmul(
                    ps_all[0:1, 0, 0, 0:1], obs_src, obs_src,
                    start=True, stop=True, tile_position=(0, 0),
                )
                if ca is not None:
                    add_dep_helper(obs.ins, ca.ins, reason="obs after carrier")
                kmax = 4 if f % 2 == 0 else 2
                last_obs = None
                for k in range(kmax):
                    cobs = nc.tensor.matmul(
                        ps_all[0:1, 0, 0, 0:1],
                        rhs[0 : k + 1, 0:1].bitcast(F32),
                        rhs[0 : k + 1, 0:1].bitcast(F32),
                        start=True, stop=True, tile_position=(0, 0),
                    )
                    if ca is not None:
                        add_dep_helper(cobs.ins, ca.ins, reason="obs after carrier")
                    last_obs = cobs

                for q in range(4):
                    Q = 4 * f + q
                    p = q % 2
                    s = 2 * f + (q // 2)
                    if q > 0:
                        ca, cv = make_carriers()

                    # One N=512 matmul per group per quarter into parity set
                    # p; the other parity's banks are being drained by the
                    # consumers concurrently (double buffering).
                    for g in range(NGROUPS):
                        col0 = q * NT
                        last_mm = nc.tensor.matmul(
                            ps_all[:, p, g, :],
                            lw_t[32 * g : 32 * g + 4,
                                 128 * s : 128 * (s + 1)],
                            rhs[32 * g : 32 * g + 4, col0 : col0 + NT],
                            start=True, stop=True,
                            tile_position=(32 * g, 0),
                        )
                        carrier = ca if g < 2 else cv
                        if carrier is not None:
                            add_dep_helper(
                                last_mm.ins, carrier.ins,
                                reason="slot WAR via carrier",
                            )
                        add_dep_helper(
                            last_mm.ins, last_obs.ins,
                            reason="mm after observers",
                        )

                    # Consumers: ACT owns groups {0,1}, DVE owns {2,3};
                    # FD=1024 over the parity-p bank set.
                    i_red = 2 * Q
                    cons_a = nc.scalar.activation(
                        out=ps_all[:, p, 0:2, :],
                        in_=ps_all[:, p, 0:2, :],
                        func=mybir.ActivationFunctionType.Relu,
                        accum_out=red[:, i_red : i_red + 1],
                    )
                    act_hist.append(cons_a.ins)
                    cons_v = nc.vector.tensor_scalar(
                        out=ps_all[:, p, 2:4, :],
                        in0=ps_all[:, p, 2:4, :],
                        scalar1=0.0,
                        scalar2=None,
                        op0=mybir.AluOpType.max,
                        op1=mybir.AluOpType.add,
                        accum_out=red[:, i_red + 1 : i_red + 2],
                    )
                    dve_hist.append(cons_v.ins)

            # Final DVE carrier observes ACT's tail so the reduce needs one
            # wait (red col 0 is ACT-written: merges into the same sem).
            cfin = nc.vector.tensor_scalar(
                out=tiny_v2[:], in0=red[:, 0:1],
                scalar1=0.0, scalar2=None, op0=mybir.AluOpType.add,
            )
            for ins in act_hist[-2:]:
                add_dep_helper(cfin.ins, ins, reason="reduce observes ACT tail")
            outs_t = singles.tile([128, 1], F32)
            rsum = nc.vector.reduce_sum(outs_t[:], red[:],
                                        axis=mybir.AxisListType.X)
            # SWDGE: separate lane space -> single wait (the reduce).
            odma = nc.gpsimd.dma_start(out=outs[:], in_=outs_t[:])

            # Pre-observe every proc on SP so the TileContext-exit drain
            # (single-wait NOP) has nothing left to wait on.
            drain_deps = [act_hist[-1], rsum.ins, last_mm.ins, odma.ins,
                          lw_dma.ins]
            drain_deps += [p.ins for p in pres]
            drain_deps += [d.ins for d in fill_dmas[-8:]]
            for dins in drain_deps:
                dr = nc.sync.drain(fusable=False)
                add_dep_helper(dr.ins, dins, reason="pre-drain observe")

    _NC_CACHE = nc
    return nc


def kernel(x, W1, b1, W2, b2, W3, b3):
    global LAST_RESULT
    x = np.asarray(x, dtype=np.float32)
    W1 = np.asarray(W1, dtype=np.float32)
    b1 = np.asarray(b1, dtype=np.float32)
    W2 = np.asarray(W2, dtype=np.float32)
    b2 = np.asarray(b2, dtype=np.float32)
    W3 = np.asarray(W3, dtype=np.float32)
    b3 = np.asarray(b3, dtype=np.float32)

    x0, x1 = x[0], x[1]
    mask = x0 != 0.0

    rows_any = mask.any(axis=1)
    cols_any = mask.any(axis=0)
    ridx = np.nonzero(rows_any)[0]
    cidx = np.nonzero(cols_any)[0]
    rmin, rmax = float(ridx[0]), float(ridx[-1])
    cmin, cmax = float(cidx[0]), float(cidx[-1])

    W12 = W1.astype(np.float64) @ W2.astype(np.float64)
    b12 = b1.astype(np.float64) @ W2.astype(np.float64) + b2
    v0 = W12[0]

    nr_all = (np.arange(H, dtype=np.float64) - rmin) / (rmax - rmin)
    nc_all = (np.arange(W, dtype=np.float64) - cmin) / (cmax - cmin)

    nc2 = np.tile(nc_all.astype(np.float32), 2)
    v2f = W12[2].astype(np.float32)
    v3f = W12[3].astype(np.float32)
    v1f = W12[1].astype(np.float32)

    ncpre = np.broadcast_to(
        np.tile(nc2, 2)[None, :], (NGROUPS, 2 * CHUNK)
    ).copy()
    onepre = np.ones((NGROUPS, 2 * CHUNK), dtype=np.float32)

    nc_prog = _build_bass()
    in_maps = []
    for c in range(N_CORES):
        shard = x[:, c * ROWS_PER_CORE : (c + 1) * ROWS_PER_CORE, :]
        xdv = np.empty((NFILLS, 2, NGROUPS, CHUNK), dtype=np.float32)
        sh = shard.reshape(2, NFILLS, NGROUPS, CHUNK)
        xdv[:, 0, :, :] = sh[0]
        xdv[:, 1, :, :] = sh[1]

        btab = (
            b12[:, None]
            + np.outer(v0, nr_all[c * ROWS_PER_CORE : (c + 1) * ROWS_PER_CORE])
        ).astype(np.float32)
        lwtv = np.zeros((128, 128 * ROW_SLOTS), dtype=np.float32)
        for g in range(NGROUPS):
            for s in range(ROW_SLOTS):
                f, half = divmod(s, 2)
                r_loc = 8 * f + 2 * g + half
                blk = slice(128 * s, 128 * (s + 1))
                lwtv[32 * g + 0, blk] = v2f
                lwtv[32 * g + 1, blk] = v3f
                lwtv[32 * g + 2, blk] = v1f
                lwtv[32 * g + 3, blk] = btab[:, r_loc]
        in_maps.append({"xd": xdv, "lwt": lwtv, "ncpre": ncpre,
                        "onepre": onepre})

    res = run_bass_kernel_spmd(
        nc_prog, in_maps, core_ids=list(range(N_CORES)), trace=TRACE
    )
    LAST_RESULT = res

    S = np.zeros(D, dtype=np.float64)
    for c in range(N_CORES):
        S += res.results[c]["outs"][:, 0].astype(np.float64)

    if not mask.all():
        zr, zc = np.nonzero(~mask)
        hz = (
            np.outer(nr_all[zr], W12[0])
            + np.outer(nc_all[zc], W12[1])
            + np.outer(x1[zr, zc].astype(np.float64), W12[3])
            + b12[None, :]
        )
        S -= np.maximum(hz, 0.0).sum(axis=0)

    wsum = float(mask.sum())
    out = (S @ W3.astype(np.float64)) / wsum + b3.astype(np.float64)
    return out.astype(np.float32)
